# revision 48
# baseline (speedup 1.0000x reference)
"""Soft-DTW layer (band-limited, gamma=1) as a Bass/Tile kernel on 8 TRN2 cores.

Problem: x [64, 512] f32, protos [32, 64] f32 -> out [64, 32, 1] f32
  out[b, f, 0] = softDTW(C[b,f]) / T, C[b,f][i,j] = (x[b,i]-protos[f,j])^2,
  Sakoe-Chiba band |i/511 - j/63| <= 0.2, out-of-band = BIG.

Algorithm (per (b,f) problem, exp-space):
  E'(i,j) = e^{a*i - D(i,j)} satisfies
    E'(i,j) = G(i,j) * (E'(i-1,j) + E'(i-1,j-1) + e^{-a} * E'(i,j-1))
  with G = e^{a - C} (0 outside band). Sweep columns j=0..63; each column's
  in-band rows live in window [8j-104, 8j+112) (216 rows). Per column just
  TWO vector-engine ops:
    w2[v] = e^{-a} * cprev[8+v] + cprev[7+v]              (STT)
    E[v]  = (w2[v] + E[v-1]) * G[v]                       (tensor_tensor_scan,
                                               op0=add, op1=mult, fp32 state)
  Every FB columns a per-problem rescale s=1/max keeps values in f32 range;
  ln(max) slots are logged once at the end and added back.

  G is produced in bf16 (rel err ~1.2e-3 vs 2e-2 budget) chunk by chunk,
  8 columns at a time, entirely on the Act engine — Square's input affine
  (bias = -p_j per partition) fuses the subtract into the squaring, so per
  column G takes just Square + Exp — with the out-of-band edges zeroed by
  strided memsets on Pool. All of it overlaps the DVE column DP of the
  previous chunk; per-chunk flag stamps (with real data deps, so the
  scheduler can't hoist them) collapse the DP's cross-engine waits into
  one two-sem touch per chunk.

Sharding: data-parallel over batch. Core c handles b in [8c, 8c+8); its 256
(b,f) problems sit as 2 groups of 128 partitions:
  partition p, group g -> b = 8c + 4g + p//32, f = p%32.
"""

import numpy as np
import ml_dtypes

import concourse.bass as bass
import concourse.bacc as bacc
import concourse.mybir as mybir
import concourse.tile as tile
from concourse.bass_utils import run_bass_kernel_spmd

T, K = 512, 64
NCORES = 8
L = 216          # column window length
GS = L           # per-group column stride (joint buffer, shared pad)
CS = L * 2 + 8   # column buffer: [g0 216 | g1 216 | 8 zero pad]
BOFF = 104       # column j covers rows [8j-104, 8j+112)
GCOL = 2 * L     # per-column G stride (both groups)
XPAD = 104 + T + 112          # padded x row length (728)
XBIG = 1.0e4                  # pad value; (XBIG-p)^2 ~ 1e8 -> exp -> 0
A = 0.75                      # rescale slope per row
FB = 8                        # feedback (renorm) every FB columns
# (FB=12 measured 3.0e-2 max rel err on HW — bf16 subnormal flush eats the
# small-path contributions between renorms; FB=8 keeps 1.2e-3)
CH = 8                        # G production chunk (columns)
EA = float(np.exp(A))
ECA = float(np.exp(-A))
F32 = mybir.dt.float32
BF16 = mybir.dt.bfloat16


def _ap(t, offset, dims):
    """Custom free-dim access pattern on tile t: dims = [[step, count], ...]
    (element units), keeping the partition dim."""
    ap = t[:, 0:1].copy()
    ap.ap = ap.ap[:1] + [[int(s), int(n)] for s, n in dims]
    ap.offset = int(offset)
    return ap


def _edge_runs(j0, j1, lo_side):
    """Maximal runs [ja, jb) of columns in [j0, j1) with constant edge width.
    lo edge: zero v in [0, lo_j), lo_j = ceil((7j+114)/63).
    hi edge: zero v in [hi_j+1, L), hi_j = floor((7j+12990)/63)."""
    runs = []
    j = j0
    while j < j1:
        if lo_side:
            w = -((7 * j + 114) // -63)          # ceil
        else:
            w = (7 * j + 12990) // 63            # floor -> first zero at w+1
        je = j
        while je < j1:
            wn = -((7 * je + 114) // -63) if lo_side else (7 * je + 12990) // 63
            if wn != w:
                break
            je += 1
        runs.append((j, je, w))
        j = je
    return runs


def build_nc(dump=False):
    nc = bacc.Bacc("TRN2")
    xs = nc.dram_tensor("xs", [8, T], BF16, kind="ExternalInput")
    pr = nc.dram_tensor("protos", [32, K], F32, kind="ExternalInput")
    out = nc.dram_tensor("out", [128, 2], F32, kind="ExternalOutput")

    with tile.TileContext(nc) as tc:
        with tc.tile_pool(name="main", bufs=1) as pool:
            x_all = pool.tile([128, 2 * XPAD], BF16)  # padded x per problem/group
            prt = pool.tile([128, K], F32)            # NEGATED protos (ACT bias)
            G = pool.tile([128, K * GCOL], BF16)      # banded G, layout (j, g, v)
            colA = pool.tile([128, CS], BF16)
            colB = pool.tile([128, CS], BF16)
            w2 = pool.tile([128, 2 * L], BF16)
            mxb = pool.tile([128, 16], F32)           # 8 feedback slots x 2
            s2 = pool.tile([128, 2], F32)
            lnmx = pool.tile([128, 16], F32)
            ef = pool.tile([128, 2], F32)
            efe = pool.tile([128, 2], mybir.dt.int32)
            eff = pool.tile([128, 2], F32)
            efm = pool.tile([128, 2], mybir.dt.int32)
            lnmant = pool.tile([128, 2], F32)
            lnef = pool.tile([128, 2], F32)
            lnS = pool.tile([128, 2], F32)
            tt = pool.tile([128, 2], F32)
            osb = pool.tile([128, 2], F32)
            scr = pool.tile([128, 8], F32)            # DVE pre-touch scratch
            acon = pool.tile([128, 1], F32)           # bias const A for Exp
            flags = pool.tile([128, 36], F32)         # per-chunk ACT/Pool stamps

            # ---- init ----
            nc.vector.memset(acon[:, :], A)
            # dummy Exp: pull the ACT function table load off the critical
            # path (it happens during the x DMA instead of before Square(0))
            nc.scalar.activation(scr[:, 4:5], acon[:, :],
                                 mybir.ActivationFunctionType.Exp)
            # x_all: only the pad strips need XBIG (the DMA fills the rest);
            # a full-tile memset would serialize in front of the DMA
            nc.vector.memset(x_all[:, 0:BOFF], XBIG)
            nc.vector.memset(x_all[:, BOFF + T:XPAD + BOFF], XBIG)
            nc.vector.memset(x_all[:, XPAD + BOFF + T:2 * XPAD], XBIG)

            # x: DRAM [8, 512] -> per-group replicated rows (partition p,
            # group g reads row 4g + p//32). The broadcast DMA is descriptor
            # bound (~90ns x 128 rows), so split it across two queues and in
            # two row-ranges each: the first 128 x-rows unlock columns 0-15
            # after ~1.5us while the rest streams in behind the DP.
            XSPL = 128
            # protos first (tiny; gates prtneg + every ACT bias)
            psrc = pr[:, :].unsqueeze(0).broadcast_to([4, 32, K])
            nc.sync.dma_start(prt[:, :], psrc)
            s0a = xs[0:4, 0:XSPL].unsqueeze(1).broadcast_to([4, 32, XSPL])
            nc.sync.dma_start(x_all[:, BOFF:BOFF + XSPL], s0a)
            s1a = xs[4:8, 0:XSPL].unsqueeze(1).broadcast_to([4, 32, XSPL])
            nc.gpsimd.dma_start(x_all[:, XPAD + BOFF:XPAD + BOFF + XSPL], s1a)
            s0b = xs[0:4, XSPL:T].unsqueeze(1).broadcast_to([4, 32, T - XSPL])
            nc.sync.dma_start(x_all[:, BOFF + XSPL:BOFF + T], s0b)
            s1b = xs[4:8, XSPL:T].unsqueeze(1).broadcast_to([4, 32, T - XSPL])
            nc.gpsimd.dma_start(x_all[:, XPAD + BOFF + XSPL:XPAD + BOFF + T], s1b)
            nc.vector.memset(colA[:, :], 0.0)
            nc.vector.memset(colB[:, :], 0.0)
            nc.vector.memset(mxb[:, :], 1.0)          # unused slots -> ln = 0
            # virtual-corner seed E'(-1,-1)=e^{-a} at row -1 of column -1
            # (column -1 window starts at row -112; row -1 -> pos 111)
            nc.vector.memset(colA[:, 111:112], ECA)
            nc.vector.memset(colA[:, GS + 111:GS + 112], ECA)
            tc.no_sync_barrier()

            # ---- chunked G production + column DP ----
            def emit_act(j0, ncols):
                # per column: C = (x_win - p_j)^2 via Square's input affine
                # (bias = -p_j per partition), then G = exp(A - C), in place
                for j in range(j0, j0 + ncols):
                    gout = _ap(G, j * GCOL, [[L, 2], [1, L]])
                    xwin = _ap(x_all, 8 * j, [[XPAD, 2], [1, L]])
                    nc.scalar.activation(gout, xwin,
                                         mybir.ActivationFunctionType.Square,
                                         bias=prt[:, j:j + 1], scale=1.0)
                    nc.scalar.activation(gout, gout,
                                         mybir.ActivationFunctionType.Exp,
                                         bias=acon[:, :], scale=-1.0)

            def emit_masks(j0, ncols, eng):
                # band edges: zero G outside [lo_j, hi_j] per column
                for ja, jb, lo in _edge_runs(j0, j0 + ncols, True):
                    eng.memset(
                        _ap(G, ja * GCOL, [[GCOL, jb - ja], [L, 2], [1, lo]]), 0.0)
                for ja, jb, hi in _edge_runs(j0, j0 + ncols, False):
                    if hi + 1 < L:
                        eng.memset(
                            _ap(G, ja * GCOL + hi + 1,
                                [[GCOL, jb - ja], [L, 2], [1, L - hi - 1]]), 0.0)

            state = {"cprev": colA, "ccur": colB, "fb_k": 0}

            def dp_cols(ja, jb):
                # joint 432-wide ops: the ~100ns/instruction ambient overhead
                # (with ACT/Pool streams active) makes 2 big ops beat 4 small
                # ones — measured 141.5us joint vs 158.6us interleaved
                for j in range(ja, jb):
                    cprev, ccur = state["cprev"], state["ccur"]
                    gcol = G[:, j * GCOL:(j + 1) * GCOL]
                    # w2 = e^-a * same_row + diag from previous column
                    nc.vector.scalar_tensor_tensor(
                        w2[:, :], cprev[:, 8:8 + 2 * L], ECA, cprev[:, 7:7 + 2 * L],
                        op0=mybir.AluOpType.mult, op1=mybir.AluOpType.add)
                    # E[v] = (w2[v] + E[v-1]) * G[v]
                    nc.vector.tensor_tensor_scan(
                        ccur[:, 0:2 * L], w2[:, :], gcol, 0.0,
                        op0=mybir.AluOpType.add, op1=mybir.AluOpType.mult)
                    if (j + 1) % FB == 0 and j < K - 1:
                        fb_k = state["fb_k"]
                        mx = mxb[:, 2 * fb_k:2 * fb_k + 2]
                        # stride-2 subsampled max: may undershoot the true
                        # max ~2x, harmless against the e^+-80 range margin
                        nc.vector.tensor_reduce(
                            mx, _ap(ccur, 0, [[GS, 2], [2, L // 2]]),
                            axis=mybir.AxisListType.X, op=mybir.AluOpType.max)
                        nc.vector.reciprocal(s2[:, :], mx)
                        for g in range(2):
                            nc.vector.tensor_scalar_mul(
                                ccur[:, g * GS:g * GS + L],
                                ccur[:, g * GS:g * GS + L], s2[:, g:g + 1])
                        state["fb_k"] = fb_k + 1
                    state["cprev"], state["ccur"] = ccur, cprev

            # Flag relay: after each chunk's ACT/Pool production, stamp a
            # flag cell on that engine; the DP touches both stamps in ONE
            # vector copy (2 sem waits) before the chunk, so the per-scan
            # G-region waits inside the chunk are already covered. The
            # stamps READ cells the chunk's last ops wrote — a dep-free
            # stamp gets hoisted ahead of the chunk by the scheduler and
            # covers nothing.
            def stamp_act(i, j_last):
                # mid-column cell: written by Exp, untouched by edge masks
                nc.scalar.copy(flags[:, 3 * i:3 * i + 1],
                               G[:, j_last * GCOL + 100:j_last * GCOL + 101])

            def stamp_pool(i, j_last):
                # first + last cell of the column: lo- and hi-edge memsets
                nc.gpsimd.tensor_copy(flags[:, 3 * i + 1:3 * i + 3],
                                      _ap(G, j_last * GCOL, [[L - 1, 2]]))

            def touch_flags(i):
                nc.vector.tensor_copy(scr[:, 5:8], flags[:, 3 * i:3 * i + 3])

            # warmup: chunk 0 in pieces of 1/2/2/3 columns so the first scan
            # starts as soon as one column of G exists
            pieces = [(0, 1), (1, 2), (3, 2), (5, 3)]
            for i, (j0, n) in enumerate(pieces):
                emit_act(j0, n)
                stamp_act(i, j0 + n - 1)
                emit_masks(j0, n, nc.gpsimd)
                stamp_pool(i, j0 + n - 1)
            emit_act(CH, CH)
            for i, (j0, n) in enumerate(pieces):
                touch_flags(i)
                dp_cols(j0, j0 + n)
            # cols 8-16 (produced during warmup), then 16-col chunks
            emit_masks(CH, CH, nc.gpsimd)
            stamp_act(4, 2 * CH - 1)
            stamp_pool(4, 2 * CH - 1)
            emit_act(2 * CH, 2 * CH)
            touch_flags(4)
            dp_cols(CH, 2 * CH)
            for k in range(3):
                j0 = 2 * CH + k * 2 * CH
                emit_masks(j0, 2 * CH, nc.gpsimd)
                stamp_act(5 + k, j0 + 2 * CH - 1)
                stamp_pool(5 + k, j0 + 2 * CH - 1)
                if k < 2:
                    emit_act(j0 + 2 * CH, 2 * CH)
                touch_flags(5 + k)
                if k == 2:
                    # mxb is complete after the j=55 renorm; run the Ln on
                    # ACT during the last columns, off the extraction tail
                    dp_cols(j0, 60)
                    nc.scalar.activation(lnmx[:, :], mxb[:, :],
                                         mybir.ActivationFunctionType.Ln)
                    dp_cols(60, j0 + 2 * CH)
                else:
                    dp_cols(j0, j0 + 2 * CH)

            last = state["cprev"]  # column 63 buffer
            # ---- extraction: D = a*511 - sum(ln mx) - ln(E'fin); out = D/512 ----
            nc.vector.tensor_copy(ef[:, 0:1], last[:, 111:112])
            nc.vector.tensor_copy(ef[:, 1:2], last[:, GS + 111:GS + 112])
            # ACT's Ln mishandles tiny args (E'fin can be ~1e-37), so do a
            # frexp-style log: ln(ef) = Ln(mantissa) + (exp - 127)*ln2.
            # (the -127*ln2 is folded into the final affine)
            eiv = ef[:, :].bitcast(mybir.dt.int32)
            nc.vector.tensor_scalar(efe[:, :], eiv, 23, None,
                                    op0=mybir.AluOpType.arith_shift_right)
            nc.vector.tensor_copy(eff[:, :], efe[:, :])   # int -> float value
            nc.vector.tensor_scalar(efm[:, :], eiv, 0x007FFFFF, 0x3F800000,
                                    op0=mybir.AluOpType.bitwise_and,
                                    op1=mybir.AluOpType.bitwise_or)
            nc.scalar.activation(lnmant[:, :], efm[:, :].bitcast(F32),
                                 mybir.ActivationFunctionType.Ln)
            nc.vector.scalar_tensor_tensor(
                lnef[:, :], eff[:, :], float(np.log(2.0)), lnmant[:, :],
                op0=mybir.AluOpType.mult, op1=mybir.AluOpType.add)
            nc.vector.tensor_reduce(
                lnS[:, :], lnmx[:, :].rearrange("p (k g) -> p g k", g=2),
                axis=mybir.AxisListType.X, op=mybir.AluOpType.add)
            nc.vector.tensor_tensor(tt[:, :], lnS[:, :], lnef[:, :],
                                    op=mybir.AluOpType.add)
            nc.vector.tensor_scalar(
                osb[:, :], tt[:, :], float(-1.0 / T),
                float((A * (T - 1) + 127.0 * np.log(2.0)) / T),
                op0=mybir.AluOpType.mult, op1=mybir.AluOpType.add)
            # split across two queues: the [128, 2] store is descriptor-bound
            nc.sync.dma_start(out[0:64, :], osb[0:64, :])
            nc.gpsimd.dma_start(out[64:128, :], osb[64:128, :])

    nc.compile()
    return nc


_NC = None


def _get_nc():
    global _NC
    if _NC is None:
        _NC = build_nc()
    return _NC


# ---- host-side dispatch -----------------------------------------------------
# Under axon, run_bass_kernel_spmd builds a fresh jax.jit closure per call
# (full retrace + executable-cache lookup each time). Build the sharded
# executable ONCE and reuse it: repeat calls then cost a single tunnel
# round-trip instead of three-plus.

_RUNNER = None


def _build_runner(nc):
    import jax
    from jax.sharding import Mesh, PartitionSpec
    from jax.experimental.shard_map import shard_map
    from concourse import bass2jax
    from concourse.bass2jax import _bass_exec_p, install_neuronx_cc_hook

    install_neuronx_cc_hook()

    partition_name = (nc.partition_id_tensor.name
                      if nc.partition_id_tensor else None)
    in_names, out_names, out_avals, zero_outs = [], [], [], []
    for alloc in nc.m.functions[0].allocations:
        if not isinstance(alloc, mybir.MemoryLocationSet):
            continue
        name = alloc.memorylocations[0].name
        if alloc.kind == "ExternalInput":
            if name != partition_name:
                in_names.append(name)
        elif alloc.kind == "ExternalOutput":
            out_names.append(name)
            shape = tuple(alloc.tensor_shape)
            dtype = mybir.dt.np(alloc.dtype)
            out_avals.append(jax.core.ShapedArray(shape, dtype))
            zero_outs.append(np.zeros(shape, dtype))
    n_params = len(in_names)
    all_names = in_names + out_names + ([partition_name] if partition_name else [])
    donate = tuple(range(n_params, n_params + len(out_avals)))

    def _body(*args):
        operands = list(args)
        if partition_name is not None:
            operands.append(bass2jax.partition_id_tensor())
        outs = _bass_exec_p.bind(
            *operands, out_avals=tuple(out_avals), in_names=tuple(all_names),
            out_names=tuple(out_names), lowering_input_output_aliases=(),
            sim_require_finite=True, sim_require_nnan=True, nc=nc)
        return tuple(outs)

    devices = jax.devices()[:NCORES]
    mesh = Mesh(np.asarray(devices), ("core",))
    in_specs = (PartitionSpec("core"),) * (n_params + len(out_avals))
    out_specs = (PartitionSpec("core"),) * len(out_names)
    sharded = jax.jit(
        shard_map(_body, mesh=mesh, in_specs=in_specs, out_specs=out_specs,
                  check_rep=False),
        donate_argnums=donate, keep_unused=True)

    def run(in_concat):
        zo = [np.zeros((NCORES * z.shape[0],) + z.shape[1:], z.dtype)
              for z in zero_outs]
        outs = sharded(*[in_concat[n] for n in in_names], *zo)
        return {n: np.asarray(outs[i]) for i, n in enumerate(out_names)}

    return run


def _get_runner():
    global _RUNNER
    if _RUNNER is None:
        _RUNNER = _build_runner(_get_nc())
    return _RUNNER


def kernel(x: np.ndarray, protos: np.ndarray) -> np.ndarray:
    xb = np.ascontiguousarray(x, dtype=np.float32).astype(ml_dtypes.bfloat16)
    pb = np.ascontiguousarray(protos, dtype=np.float32).astype(ml_dtypes.bfloat16)
    try:
        pneg = np.ascontiguousarray(-protos, dtype=np.float32)
        r = _get_runner()({"xs": xb, "protos": np.tile(pneg, (NCORES, 1))})
        res = r["out"].reshape(NCORES, 128, 2)
    except Exception:
        nc = _get_nc()
        pneg = np.ascontiguousarray(-protos, dtype=np.float32)
        in_maps = [{"xs": xb[8 * c: 8 * c + 8], "protos": pneg}
                   for c in range(NCORES)]
        rr = run_bass_kernel_spmd(nc, in_maps, core_ids=list(range(NCORES)))
        res = np.stack([rr.results[c]["out"] for c in range(NCORES)])
    out = np.empty((64, 32, 1), dtype=np.float32)
    for c in range(NCORES):
        blk = res[c].reshape(4, 32, 2).transpose(2, 0, 1)  # [g, bb, f]
        out[8 * c: 8 * c + 8, :, 0] = blk.reshape(8, 32)
    return out


if __name__ == "__main__":
    x = np.load("/root/problem/x.npy")
    protos = np.load("/root/problem/protos.npy")
    got = kernel(x, protos)
    D_true = np.load("/root/problem/D_true.npy").reshape(64, 32) / T
    rel = np.abs(got[:, :, 0] - D_true) / np.abs(D_true)
    print("rel err max", rel.max(), "mean", rel.mean())
